# revision 6
# baseline (speedup 1.0000x reference)
"""LIFSpike Trainium2 kernel (Bass/Tile), SPMD over 8 NeuronCores.

Reference semantics (T=4, tau=2, vth=1, vreset=0, decay_input=False,
hard reset):
    xs = x.reshape(T, B//T, C, H, W)
    v0 = 0
    h_t = v_t * 0.5 + x_t
    s_t = (h_t >= 1.0)
    v_{t+1} = h_t * (h_t < 1.0)
    out = s.reshape(B, C, H, W)

Kernel-side reformulation (exact in fp32 -- all rescalings are by powers
of two, which commute with fp rounding):
    r_t := 2^t * h_t,  host supplies x'_t = 2^t * x_t
    r_0     = x'_0                                   (plain DMA load)
    s_t     = (r_t >= 2^t)
    q_t     = (r_t < 2^t) * r_t   (= 2^t * v_{t+1})  (one STT op, DVE)
    r_{t+1} = q_t + x'_{t+1}                         (tensor add, in place
                                                      on the loaded x' tile)

The spike comparison runs on the otherwise-idle Scalar (ACT) engine as
    sign(r_t + bias_t),  bias_t = nextafter(-2^t, 0)
which is -1/0/+1 in fp8; the host decodes s = (value > 0).  This is
exact: r + bias == 0 only for r == 2^t*(1-2^-24) (the largest f32 below
the threshold), and sign(0)=0 decodes to s=0, which is correct.

All 16 MiB of input streams through plain HWDGE loads with no compute
dependency (2 MiB each), so DMA -- the roofline resource -- never
stalls.  Engine budget per core: DVE 12 STT + 4 adds ~37us, Pool 8 adds
~34us, ACT 16 signs ~32us, DMA ~59us (bound).

Host-side input layout per core (partition-major, t-major):
    x_core[p, t*8192 + b*2048 + j] = 2^t * x[t*32 + core*4 + b, flat=p*2048+j]
Output layout is b-major so each chain stores once, contiguously:
    s_core[p, b*8192 + t*2048 + j]
"""

import numpy as np

T = 4
BP = 32               # B // T
NCORES = 8
BPC = BP // NCORES    # chains per core = 4
SLICE = 256 * 32 * 32  # elements per (t, b) slice = 262144
P = 128
W = SLICE // P        # free elems per chain-timestep tile = 2048
FREE_T = BPC * W      # 8192 (one timestep slab, all chains)
FREE = T * FREE_T     # 32768

_cache = {}


def _build_program():
    import concourse.bass as bass
    import concourse.tile as tile
    from concourse import bacc, mybir

    Alu = mybir.AluOpType
    Act = mybir.ActivationFunctionType
    f32 = mybir.dt.float32
    out_dt = mybir.dt.float8e4

    nc = bacc.Bacc(debug=False)
    x = nc.dram_tensor("x", [P, FREE], f32, kind="ExternalInput").ap()
    s = nc.dram_tensor("s", [P, FREE], out_dt, kind="ExternalOutput").ap()

    with tile.TileContext(nc) as tc:
        with (
            tc.tile_pool(name="state", bufs=1) as vpool,
            tc.tile_pool(name="sout", bufs=1) as spool,
        ):
            # one resident slab per timestep; x' loads land here and the
            # chain state r_t is updated in place (r_{t+1} tile = x'_{t+1}
            # tile += q_t)
            slabs = [
                vpool.tile([P, FREE_T], f32, tag=f"xs{t}", name=f"xs{t}")
                for t in range(T)
            ]
            # per-chain reset scratch q_t (reused across timesteps)
            qts = [
                vpool.tile([P, W], f32, tag=f"q{b}", name=f"q{b}")
                for b in range(BPC)
            ]
            outs = [
                spool.tile([P, T * W], out_dt, tag=f"s{b}", name=f"sout{b}")
                for b in range(BPC)
            ]
            # per-timestep sign biases as [128,1] const tiles
            biases = vpool.tile([P, T], f32, tag="bias", name="bias")
            for t in range(T):
                bias = float(np.nextafter(np.float32(-(1 << t)), np.float32(0)))
                nc.gpsimd.memset(biases[:, t:t + 1], bias)

            # stream all input up front: 2 MiB per DMA (2 chains), no
            # compute dependencies
            for t in range(T):
                for half in range(2):
                    lo = t * FREE_T + half * (2 * W)
                    nc.sync.dma_start(
                        slabs[t][:, half * 2 * W:(half + 1) * 2 * W],
                        x[:, lo:lo + 2 * W],
                    )

            for t in range(T):
                th = float(1 << t)
                for b in range(BPC):
                    p = slabs[t][:, b * W:(b + 1) * W]
                    if t < T - 1:
                        q = qts[b]
                        nc.vector.scalar_tensor_tensor(
                            q[:], p, th, p, Alu.is_lt, Alu.mult
                        )
                        nxt = slabs[t + 1][:, b * W:(b + 1) * W]
                        # r_{t+1} = x'_{t+1} + q_t, in place on the slab.
                        # t3 adds ride on DVE (keeps pace with the final
                        # loads); earlier adds go to the Pool engine.
                        eng = nc.vector if t == T - 2 else nc.gpsimd
                        eng.tensor_tensor(nxt, nxt, q[:], Alu.add)
                    # spike: sign(r + bias) on ACT, fp8 out, decode >0 host-side
                    nc.scalar.activation(
                        outs[b][:, t * W:(t + 1) * W], p, Act.Sign,
                        bias=biases[:, t:t + 1],
                    )
                    if t == T - 1:
                        nc.scalar.dma_start(
                            s[:, b * T * W:(b + 1) * T * W], outs[b][:]
                        )
    nc.compile()
    return nc


def _shard(x):
    # x: (128, 256, 32, 32) f32 -> list of 8 per-core [128, 32768] arrays,
    # timestep t pre-scaled by 2^t (exact in fp32)
    xr = np.ascontiguousarray(x).reshape(T, BP, SLICE)
    tscale = (2.0 ** np.arange(T, dtype=np.float32)).astype(np.float32)
    shards = []
    for k in range(NCORES):
        xk = xr[:, k * BPC:(k + 1) * BPC, :].reshape(T, BPC, P, W)
        xk = xk * tscale[:, None, None, None]
        xk = xk.transpose(2, 0, 1, 3).reshape(P, FREE)
        shards.append(np.asarray(xk, dtype=np.float32))
    return shards


def _unshard(parts):
    # parts: 8 per-core [128, 32768] arrays (fp8 sign values, b-major)
    # -> (128,256,32,32) f32 spikes; spike iff stored value > 0
    out = np.empty((T, BP, SLICE), dtype=np.float32)
    for k, sk in enumerate(parts):
        sk = (np.asarray(sk).astype(np.float32) > 0).astype(np.float32)
        sk = sk.reshape(P, BPC, T, W)
        out[:, k * BPC:(k + 1) * BPC, :] = (
            sk.transpose(2, 1, 0, 3).reshape(T, BPC, SLICE)
        )
    return out.reshape(T * BP, 256, 32, 32)


def kernel(x):
    from concourse.bass_utils import run_bass_kernel_spmd

    if "nc" not in _cache:
        _cache["nc"] = _build_program()
    nc = _cache["nc"]

    shards = _shard(np.asarray(x, dtype=np.float32))
    in_maps = [{"x": sk} for sk in shards]
    res = run_bass_kernel_spmd(nc, in_maps, list(range(NCORES)))
    return _unshard([res.results[k]["s"] for k in range(NCORES)])
